# revision 8
# baseline (speedup 1.0000x reference)
"""CMG MoE-routing kernel for Trainium2 (8 NeuronCores, data-parallel on batch).

Reference computation (per sample b):
  x = concat(motion, command)                      # [B, 576]
  g = elu(x@g_w1+g_b1); g = elu(g@g_w2+g_b2)
  coeffs = softmax(g@g_w3+g_b3)                    # [B, 8]
  for l in 0..5: x = sum_e coeffs[:,e]*(x@W_l[e]+b_l[e]); elu between layers
  out = x                                          # [B, 512]

Device strategy (per core, B_local = 1024):
  - Activations live transposed in SBUF: xT[dim, B] as [128, kt, B] tiles.
    Host pre-transposes/pads/tiles inputs, post-transposes the output.
  - All matmuls in fp32r (full PE rate at N=512, ~1.5e-4 rel err).
  - softmax over the 8 experts runs in the transposed layout using small PE
    matmuls for the cross-partition sum and the broadcast back to 128 rows.
  - MoE layer: y = sum_e (coeffs_e * xT) @ W_e + blended bias, where
    coeffs_e * xT is a DVE multiply against a PE-replicated coeff tile and
    experts accumulate into SBUF. The first expert's PSUM group also carries
    the blended-bias matmul (bias_stack.T @ coeffs, K=8).
  - Inter-layer activations store elu(y)+1; the -1 is folded into the next
    layer's bias host-side (b' = b - colsum(W)), saving a DVE pass per tile.
"""
import sys
sys.path.insert(0, "/opt/trn_rl_repo")

import numpy as np

B = 8192
N_CORES = 8
B_LOC = B // N_CORES          # 1024
MOTION = 512
COMMAND = 64
IN_DIM = MOTION + COMMAND     # 576
IN_PAD = 640                  # 5 * 128
HID = 1024
E = 8
OUT = 512
P = 128
NCH = 2                       # batch chunks per matmul (N = B_LOC / NCH = 512)
CH = B_LOC // NCH

LAYER_KT = [IN_PAD // P, 8, 8, 8, 8, 8]
LAYER_MT = [8, 8, 8, 8, 8, OUT // P]

_CACHED = None


def _build_program():
    import concourse.tile as tile
    from concourse import mybir, bacc

    f32 = mybir.dt.float32
    f32r = mybir.dt.float32r
    ACT = mybir.ActivationFunctionType
    ALU = mybir.AluOpType

    nc = bacc.Bacc("TRN2", target_bir_lowering=False, debug=False)

    # ---- DRAM I/O (host-pre-tiled; every DMA contiguous) -------------------
    xt_d = nc.dram_tensor("xt", [P, IN_PAD // P, B_LOC], f32r, kind="ExternalInput")
    gw1_d = nc.dram_tensor("gw1", [HID // P, P, IN_PAD // P, P], f32r, kind="ExternalInput")
    gw2_d = nc.dram_tensor("gw2", [HID // P, P, HID // P, P], f32r, kind="ExternalInput")
    gw3_d = nc.dram_tensor("gw3", [P, HID // P, E], f32r, kind="ExternalInput")
    gb1_d = nc.dram_tensor("gb1", [P, HID // P], f32, kind="ExternalInput")
    gb2_d = nc.dram_tensor("gb2", [P, HID // P], f32, kind="ExternalInput")
    gb3_d = nc.dram_tensor("gb3", [E, 1], f32, kind="ExternalInput")
    w_d, b_d = [], []
    for l in range(6):
        kt, mt = LAYER_KT[l], LAYER_MT[l]
        w_d.append(nc.dram_tensor(f"w{l}", [E, mt, P, kt, P], f32r, kind="ExternalInput"))
        b_d.append(nc.dram_tensor(f"b{l}", [E, mt * P], f32r, kind="ExternalInput"))
    basis_d = nc.dram_tensor("basis", [E, E, P], f32r, kind="ExternalInput")
    ones_d = nc.dram_tensor("ones", [E, E], f32r, kind="ExternalInput")
    out_d = nc.dram_tensor("out", [P, OUT // P, B_LOC], f32, kind="ExternalOutput")

    with tile.TileContext(nc) as tc:
        with tc.tile_pool(name="xp", bufs=1) as xp, \
             tc.tile_pool(name="xe", bufs=2) as xe_pool, \
             tc.tile_pool(name="yp", bufs=1) as yp, \
             tc.tile_pool(name="cp", bufs=1) as cp, \
             tc.tile_pool(name="wt", bufs=3) as wt_pool, \
             tc.tile_pool(name="sm", bufs=1) as sm, \
             tc.tile_pool(name="et", bufs=1) as et, \
             tc.tile_pool(name="ps", bufs=3, space="PSUM") as ps, \
             tc.tile_pool(name="ps2", bufs=1, space="PSUM") as ps2:

            def mm_dense(lhsT_col, rhs_3d, kt, psum, m_rows=P, first_open=False):
                """psum[:m_rows, chunk] (+)= sum_k lhsT_col[:,k,:m_rows].T @ rhs chunk"""
                for k in range(kt):
                    for c in range(NCH):
                        nc.tensor.matmul(
                            psum[:m_rows, c * CH:(c + 1) * CH],
                            lhsT_col[:, k, :m_rows],
                            rhs_3d[:, k, c * CH:(c + 1) * CH],
                            start=(k == 0 and not first_open),
                            stop=(k == kt - 1),
                        )

            def elu1_evict(src_ap, dst_ap):
                """dst = elu(src)+1 ; src fp32 [P, B_LOC] (SBUF or PSUM)."""
                r = et.tile([P, B_LOC], f32, tag="elu_r")
                u = et.tile([P, B_LOC], f32, tag="elu_u")
                r2 = et.tile([P, B_LOC], f32, tag="elu_r2")
                nc.scalar.activation(r[:], src_ap, ACT.Relu, scale=-1.0)
                nc.scalar.activation(u[:], r[:], ACT.Exp, scale=-1.0)
                nc.scalar.activation(r2[:], src_ap, ACT.Relu)
                nc.vector.tensor_tensor(dst_ap, u[:], r2[:], ALU.add)

            def elu1_evict_bias(psum, bias_col, nbias_col, dst_ap):
                """dst = elu(psum + bias)+1 ; bias per-partition [P,1]."""
                r = et.tile([P, B_LOC], f32, tag="elu_r")
                u = et.tile([P, B_LOC], f32, tag="elu_u")
                r2 = et.tile([P, B_LOC], f32, tag="elu_r2")
                nc.scalar.activation(r[:], psum[:], ACT.Relu, scale=-1.0, bias=nbias_col)
                nc.scalar.activation(u[:], r[:], ACT.Exp, scale=-1.0)
                nc.scalar.activation(r2[:], psum[:], ACT.Relu, bias=bias_col)
                nc.vector.tensor_tensor(dst_ap, u[:], r2[:], ALU.add)

            # ---- input activations ----------------------------------------
            kt0 = IN_PAD // P
            xt = xp.tile([P, 8, B_LOC], f32r, tag="xt")
            nc.sync.dma_start(xt[:, :kt0, :], xt_d.ap())

            # ---- gating network -------------------------------------------
            def dense_layer(w_dram, bias_dram, kt, rhs, out_tile):
                bias_sb = et.tile([P, 8], f32, tag="gbias")
                nbias_sb = et.tile([P, 8], f32, tag="gnbias")
                nc.sync.dma_start(bias_sb[:], bias_dram.ap())
                nc.vector.tensor_scalar(nbias_sb[:], bias_sb[:], -1.0, None, ALU.mult)
                for m in range(HID // P):
                    wt = wt_pool.tile([P, 8, P], f32r, tag="wt")
                    nc.sync.dma_start(wt[:, :kt, :], w_dram.ap()[m])
                    psum = ps.tile([P, B_LOC], f32, tag="ps")
                    mm_dense(wt, rhs, kt, psum)
                    elu1_evict_bias(psum, bias_sb[:, m:m + 1], nbias_sb[:, m:m + 1],
                                    out_tile[:, m, :])

            g1 = xe_pool.tile([P, 8, B_LOC], f32r, tag="xe")
            dense_layer(gw1_d, gb1_d, kt0, xt, g1)
            g2 = xe_pool.tile([P, 8, B_LOC], f32r, tag="xe")
            dense_layer(gw2_d, gb2_d, HID // P, g1, g2)

            # logits: [E, B] = gw3.T @ g2
            gw3_sb = sm.tile([P, 8, E], f32r, tag="gw3")
            nc.sync.dma_start(gw3_sb[:], gw3_d.ap())
            ps_log = ps.tile([P, B_LOC], f32, tag="ps")
            mm_dense(gw3_sb, g2, HID // P, ps_log, m_rows=E)

            # softmax over partitions 0..7
            gb3_sb = sm.tile([E, 1], f32, tag="gb3")
            nc.sync.dma_start(gb3_sb[:], gb3_d.ap())
            ex = sm.tile([E, B_LOC], f32r, tag="ex")
            nc.scalar.activation(ex[:], ps_log[:E, :], ACT.Exp, bias=gb3_sb[:])
            ones_sb = sm.tile([E, E], f32r, tag="ones")
            nc.sync.dma_start(ones_sb[:], ones_d.ap())
            ones8 = ones_sb[:, 0:1]
            ps_den = ps2.tile([P, B_LOC], f32, tag="ps2")
            for c in range(NCH):
                nc.tensor.matmul(ps_den[:1, c * CH:(c + 1) * CH], ones8,
                                 ex[:, c * CH:(c + 1) * CH], start=True, stop=True)
            recip = sm.tile([1, B_LOC], f32r, tag="recip")
            with nc.allow_low_precision(reason="f32r is fp32-width; rounding only"):
                nc.vector.reciprocal(recip[:], ps_den[:1, :])
            ones1x8 = ones_sb[0:1, :]
            ps_rb = ps2.tile([P, B_LOC], f32, tag="ps2")
            for c in range(NCH):
                nc.tensor.matmul(ps_rb[:E, c * CH:(c + 1) * CH], ones1x8,
                                 recip[:, c * CH:(c + 1) * CH], start=True, stop=True)
            coeffs = sm.tile([E, B_LOC], f32r, tag="coeffs")
            nc.vector.tensor_tensor(coeffs[:], ex[:], ps_rb[:E, :], ALU.mult)

            # replicate each coeff row across 128 partitions: C[:, e, :]
            basis = sm.tile([E, E, P], f32r, tag="basis")
            nc.sync.dma_start(basis[:], basis_d.ap())
            cmat = cp.tile([P, E, B_LOC], f32, tag="C")
            for e in range(E):
                ps_c = ps2.tile([P, B_LOC], f32, tag="ps2")
                for c in range(NCH):
                    nc.tensor.matmul(ps_c[:, c * CH:(c + 1) * CH], basis[:, e, :],
                                     coeffs[:, c * CH:(c + 1) * CH], start=True, stop=True)
                nc.scalar.activation(cmat[:, e, :], ps_c[:], ACT.Copy)

            # ---- MoE stack -------------------------------------------------
            cur = xt
            for l in range(6):
                kt, mt = LAYER_KT[l], LAYER_MT[l]
                bst = sm.tile([E, 8 * P], f32r, tag="bst")
                nc.sync.dma_start(bst[:, :mt * P], b_d[l].ap())
                y = yp.tile([P, 8, B_LOC], f32, tag="y")
                for e in range(E):
                    xe = xe_pool.tile([P, 8, B_LOC], f32r, tag="xe")
                    for k in range(kt):
                        nc.vector.tensor_tensor(xe[:, k, :], cur[:, k, :],
                                                cmat[:, e, :], ALU.mult)
                    for m in range(mt):
                        wt = wt_pool.tile([P, 8, P], f32r, tag="wt")
                        nc.sync.dma_start(wt[:, :kt, :], w_d[l].ap()[e, m])
                        psum = ps.tile([P, B_LOC], f32, tag="ps")
                        if e == 0:
                            for c in range(NCH):
                                nc.tensor.matmul(psum[:, c * CH:(c + 1) * CH],
                                                 bst[:, m * P:(m + 1) * P],
                                                 coeffs[:, c * CH:(c + 1) * CH],
                                                 start=True, stop=False)
                            mm_dense(wt, xe, kt, psum, first_open=True)
                            nc.scalar.activation(y[:, m, :], psum[:], ACT.Copy)
                        else:
                            mm_dense(wt, xe, kt, psum)
                            nc.vector.tensor_tensor(y[:, m, :], psum[:], y[:, m, :],
                                                    ALU.add)
                if l < 5:
                    nxt = xp.tile([P, 8, B_LOC], f32r, tag="xt")
                    for m in range(mt):
                        elu1_evict(y[:, m, :], nxt[:, m, :])
                    cur = nxt
                else:
                    nc.sync.dma_start(out_d.ap(), y[:, :mt, :])

    nc.compile()
    return nc


def _prep_w(w, pad_to=None):
    """[din, dout] -> [mt, P, kt, P] contiguous lhsT tiles (din padded)."""
    din, dout = w.shape
    if pad_to is not None and pad_to != din:
        wp = np.zeros((pad_to, dout), np.float32)
        wp[:din] = w
        w, din = wp, pad_to
    kt, mt = din // P, dout // P
    return np.ascontiguousarray(
        w.reshape(kt, P, mt, P).transpose(2, 1, 0, 3), dtype=np.float32)


def _prep_we(w, pad_to=None):
    """[E, din, dout] -> [E, mt, P, kt, P]."""
    e, din, dout = w.shape
    if pad_to is not None and pad_to != din:
        wp = np.zeros((e, pad_to, dout), np.float32)
        wp[:, :din] = w
        w, din = wp, pad_to
    kt, mt = din // P, dout // P
    return np.ascontiguousarray(
        w.reshape(e, kt, P, mt, P).transpose(0, 3, 2, 1, 4), dtype=np.float32)


def _make_in_maps(inputs):
    motion = np.asarray(inputs["motion"], np.float32)
    command = np.asarray(inputs["command"], np.float32)

    gw2 = np.asarray(inputs["g_w2"], np.float32)
    gw3 = np.asarray(inputs["g_w3"], np.float32)
    shared = {
        "gw1": _prep_w(np.asarray(inputs["g_w1"], np.float32), pad_to=IN_PAD),
        "gw2": _prep_w(gw2),
        "gw3": np.ascontiguousarray(gw3.reshape(HID // P, P, E).transpose(1, 0, 2)),
        # inter-layer activations carry elu(z)+1; fold the -1 into next biases
        "gb1": np.ascontiguousarray(np.asarray(inputs["g_b1"], np.float32).reshape(HID // P, P).T),
        "gb2": np.ascontiguousarray(
            (np.asarray(inputs["g_b2"], np.float32) - gw2.sum(0)).reshape(HID // P, P).T),
        "gb3": np.ascontiguousarray(
            (np.asarray(inputs["g_b3"], np.float32) - gw3.sum(0)).reshape(E, 1)),
    }
    for l in range(6):
        w = np.asarray(inputs[f"w{l}"], np.float32)
        bias = np.asarray(inputs[f"b{l}"], np.float32).copy()
        if l > 0:
            bias -= w.sum(axis=1)
        shared[f"w{l}"] = _prep_we(w, pad_to=IN_PAD if l == 0 else None)
        shared[f"b{l}"] = np.ascontiguousarray(bias)

    basis_np = np.zeros((E, E, P), np.float32)
    for e in range(E):
        basis_np[e, e, :] = 1.0
    shared["basis"] = basis_np
    shared["ones"] = np.ones((E, E), np.float32)

    x_cat = np.concatenate([motion, command], axis=1)
    x_pad = np.zeros((B, IN_PAD), np.float32)
    x_pad[:, :IN_DIM] = x_cat
    in_maps = []
    for c in range(N_CORES):
        xs = x_pad[c * B_LOC:(c + 1) * B_LOC]
        xt = np.ascontiguousarray(
            xs.T.reshape(IN_PAD // P, P, B_LOC).transpose(1, 0, 2))
        in_maps.append({"xt": xt, **shared})
    return in_maps


def _assemble_out(core_outs):
    outs = []
    for o in core_outs:                                    # [P, OUT/P, B_LOC]
        outs.append(o.transpose(2, 1, 0).reshape(B_LOC, OUT))
    return np.concatenate(outs, axis=0).astype(np.float32)


def kernel(**inputs):
    global _CACHED
    from concourse import bass_utils

    if _CACHED is None:
        _CACHED = _build_program()
    nc = _CACHED

    in_maps = _make_in_maps(inputs)
    res = bass_utils.run_bass_kernel_spmd(
        nc, in_maps, core_ids=list(range(N_CORES)), trace=False)
    return _assemble_out([res.results[c]["out"] for c in range(N_CORES)])


# revision 9
# speedup vs baseline: 1.0146x; 1.0146x over previous
"""CMG MoE-routing kernel for Trainium2 (8 NeuronCores, data-parallel on batch).

Reference computation (per sample b):
  x = concat(motion, command)                      # [B, 576]
  g = elu(x@g_w1+g_b1); g = elu(g@g_w2+g_b2)
  coeffs = softmax(g@g_w3+g_b3)                    # [B, 8]
  for l in 0..5: x = sum_e coeffs[:,e]*(x@W_l[e]+b_l[e]); elu between layers
  out = x                                          # [B, 512]

Device strategy (per core, B_local = 1024):
  - Activations live transposed in SBUF: xT[dim, B] as [128, kt, B] tiles.
    Host pre-transposes/pads/tiles inputs, post-transposes the output.
  - All matmuls in fp32r (full PE rate at N=512, ~1.5e-4 rel err).
  - softmax over the 8 experts runs in the transposed layout using small PE
    matmuls for the cross-partition sum and the broadcast back to 128 rows.
  - MoE layer: y = sum_e (coeffs_e * xT) @ W_e + blended bias, where
    coeffs_e * xT is a DVE multiply against a PE-replicated coeff tile and
    experts accumulate into SBUF. The first expert's PSUM group also carries
    the blended-bias matmul (bias_stack.T @ coeffs, K=8).
  - Inter-layer activations store elu(y)+1; the -1 is folded into the next
    layer's bias host-side (b' = b - colsum(W)), saving a DVE pass per tile.
"""
import sys
sys.path.insert(0, "/opt/trn_rl_repo")

import numpy as np

B = 8192
N_CORES = 8
B_LOC = B // N_CORES          # 1024
MOTION = 512
COMMAND = 64
IN_DIM = MOTION + COMMAND     # 576
IN_PAD = 640                  # 5 * 128
HID = 1024
E = 8
OUT = 512
P = 128
NCH = 2                       # batch chunks per matmul (N = B_LOC / NCH = 512)
CH = B_LOC // NCH

LAYER_KT = [IN_PAD // P, 8, 8, 8, 8, 8]
LAYER_MT = [8, 8, 8, 8, 8, OUT // P]

_CACHED = None


def _build_program():
    import concourse.tile as tile
    from concourse import mybir, bacc

    f32 = mybir.dt.float32
    f32r = mybir.dt.float32r
    ACT = mybir.ActivationFunctionType
    ALU = mybir.AluOpType

    nc = bacc.Bacc("TRN2", target_bir_lowering=False, debug=False)

    # ---- DRAM I/O (host-pre-tiled; every DMA contiguous) -------------------
    xt_d = nc.dram_tensor("xt", [P, IN_PAD // P, B_LOC], f32r, kind="ExternalInput")
    gw1_d = nc.dram_tensor("gw1", [HID // P, P, IN_PAD // P, P], f32r, kind="ExternalInput")
    gw2_d = nc.dram_tensor("gw2", [HID // P, P, HID // P, P], f32r, kind="ExternalInput")
    gw3_d = nc.dram_tensor("gw3", [P, HID // P, E], f32r, kind="ExternalInput")
    gb1_d = nc.dram_tensor("gb1", [P, HID // P], f32, kind="ExternalInput")
    gb2_d = nc.dram_tensor("gb2", [P, HID // P], f32, kind="ExternalInput")
    gb3_d = nc.dram_tensor("gb3", [E, 1], f32, kind="ExternalInput")
    w_d, b_d = [], []
    for l in range(6):
        kt, mt = LAYER_KT[l], LAYER_MT[l]
        w_d.append(nc.dram_tensor(f"w{l}", [E, mt, P, kt, P], f32r, kind="ExternalInput"))
        b_d.append(nc.dram_tensor(f"b{l}", [E, mt * P], f32r, kind="ExternalInput"))
    basis_d = nc.dram_tensor("basis", [E, E, P], f32r, kind="ExternalInput")
    ones_d = nc.dram_tensor("ones", [E, E], f32r, kind="ExternalInput")
    out_d = nc.dram_tensor("out", [P, OUT // P, B_LOC], f32, kind="ExternalOutput")

    with tile.TileContext(nc) as tc:
        with tc.tile_pool(name="xp", bufs=1) as xp, \
             tc.tile_pool(name="xe", bufs=2) as xe_pool, \
             tc.tile_pool(name="yp", bufs=1) as yp, \
             tc.tile_pool(name="cp", bufs=1) as cp, \
             tc.tile_pool(name="wt", bufs=4) as wt_pool, \
             tc.tile_pool(name="sm", bufs=1) as sm, \
             tc.tile_pool(name="et", bufs=1) as et, \
             tc.tile_pool(name="ps", bufs=3, space="PSUM") as ps, \
             tc.tile_pool(name="ps2", bufs=1, space="PSUM") as ps2:

            def mm_dense(lhsT_col, rhs_3d, kt, psum, m_rows=P, first_open=False):
                """psum[:m_rows, chunk] (+)= sum_k lhsT_col[:,k,:m_rows].T @ rhs chunk"""
                for k in range(kt):
                    for c in range(NCH):
                        nc.tensor.matmul(
                            psum[:m_rows, c * CH:(c + 1) * CH],
                            lhsT_col[:, k, :m_rows],
                            rhs_3d[:, k, c * CH:(c + 1) * CH],
                            start=(k == 0 and not first_open),
                            stop=(k == kt - 1),
                        )

            def elu1_evict(src_ap, dst_ap):
                """dst = elu(src) ; src fp32 [P, B_LOC] SBUF."""
                r = et.tile([P, B_LOC], f32, tag="elu_r")
                u = et.tile([P, B_LOC], f32, tag="elu_u")
                v = et.tile([P, B_LOC], f32, tag="elu_v")
                nc.scalar.activation(r[:], src_ap, ACT.Relu, scale=-1.0)
                nc.scalar.activation(u[:], r[:], ACT.Exp, scale=-1.0)
                nc.vector.tensor_scalar(v[:], src_ap, 0.0, 1.0, ALU.max, ALU.subtract)
                nc.vector.tensor_tensor(dst_ap, u[:], v[:], ALU.add)

            def elu1_evict_bias(psum, bias_col, nbias_col, dst_ap):
                """dst = elu(psum + bias)+1 ; bias per-partition [P,1]."""
                r = et.tile([P, B_LOC], f32, tag="elu_r")
                u = et.tile([P, B_LOC], f32, tag="elu_u")
                r2 = et.tile([P, B_LOC], f32, tag="elu_r2")
                nc.scalar.activation(r[:], psum[:], ACT.Relu, scale=-1.0, bias=nbias_col)
                nc.scalar.activation(u[:], r[:], ACT.Exp, scale=-1.0)
                nc.scalar.activation(r2[:], psum[:], ACT.Relu, bias=bias_col)
                nc.vector.tensor_tensor(dst_ap, u[:], r2[:], ALU.add)

            # ---- input activations ----------------------------------------
            kt0 = IN_PAD // P
            xt = xp.tile([P, 8, B_LOC], f32r, tag="xt")
            for k in range(kt0):
                nc.sync.dma_start(xt[:, k, :], xt_d.ap()[:, k, :])

            # ---- gating network -------------------------------------------
            def dense_layer(w_dram, bias_dram, kt, rhs, out_tile):
                bias_sb = et.tile([P, 8], f32, tag="gbias")
                nbias_sb = et.tile([P, 8], f32, tag="gnbias")
                nc.sync.dma_start(bias_sb[:], bias_dram.ap())
                nc.vector.tensor_scalar(nbias_sb[:], bias_sb[:], -1.0, None, ALU.mult)
                for m in range(HID // P):
                    wt = wt_pool.tile([P, 8, P], f32r, tag="wt")
                    nc.sync.dma_start(wt[:, :kt, :], w_dram.ap()[m])
                    psum = ps.tile([P, B_LOC], f32, tag="ps")
                    mm_dense(wt, rhs, kt, psum)
                    elu1_evict_bias(psum, bias_sb[:, m:m + 1], nbias_sb[:, m:m + 1],
                                    out_tile[:, m, :])

            g1 = xe_pool.tile([P, 8, B_LOC], f32r, tag="xe")
            dense_layer(gw1_d, gb1_d, kt0, xt, g1)
            g2 = xe_pool.tile([P, 8, B_LOC], f32r, tag="xe")
            dense_layer(gw2_d, gb2_d, HID // P, g1, g2)

            # logits: [E, B] = gw3.T @ g2
            gw3_sb = sm.tile([P, 8, E], f32r, tag="gw3")
            nc.sync.dma_start(gw3_sb[:], gw3_d.ap())
            ps_log = ps.tile([P, B_LOC], f32, tag="ps")
            mm_dense(gw3_sb, g2, HID // P, ps_log, m_rows=E)

            # softmax over partitions 0..7
            gb3_sb = sm.tile([E, 1], f32, tag="gb3")
            nc.sync.dma_start(gb3_sb[:], gb3_d.ap())
            ex = et.tile([E, B_LOC], f32r, tag="elu_r")
            nc.scalar.activation(ex[:], ps_log[:E, :], ACT.Exp, bias=gb3_sb[:])
            ones_sb = sm.tile([E, E], f32r, tag="ones")
            nc.sync.dma_start(ones_sb[:], ones_d.ap())
            ones8 = ones_sb[:, 0:1]
            ps_den = ps2.tile([P, B_LOC], f32, tag="ps2")
            for c in range(NCH):
                nc.tensor.matmul(ps_den[:1, c * CH:(c + 1) * CH], ones8,
                                 ex[:, c * CH:(c + 1) * CH], start=True, stop=True)
            recip = et.tile([1, B_LOC], f32r, tag="elu_u")
            with nc.allow_low_precision(reason="f32r is fp32-width; rounding only"):
                nc.vector.reciprocal(recip[:], ps_den[:1, :])
            ones1x8 = ones_sb[0:1, :]
            ps_rb = ps2.tile([P, B_LOC], f32, tag="ps2")
            for c in range(NCH):
                nc.tensor.matmul(ps_rb[:E, c * CH:(c + 1) * CH], ones1x8,
                                 recip[:, c * CH:(c + 1) * CH], start=True, stop=True)
            coeffs = sm.tile([E, B_LOC], f32r, tag="coeffs")
            nc.vector.tensor_tensor(coeffs[:], ex[:], ps_rb[:E, :], ALU.mult)

            # replicate each coeff row across 128 partitions: C[:, e, :]
            basis = et.tile([E, E, P], f32r, tag="elu_r2")
            nc.sync.dma_start(basis[:], basis_d.ap())
            cmat = cp.tile([P, E, B_LOC], f32, tag="C")
            for e in range(E):
                ps_c = ps2.tile([P, B_LOC], f32, tag="ps2")
                for c in range(NCH):
                    nc.tensor.matmul(ps_c[:, c * CH:(c + 1) * CH], basis[:, e, :],
                                     coeffs[:, c * CH:(c + 1) * CH], start=True, stop=True)
                nc.scalar.activation(cmat[:, e, :], ps_c[:], ACT.Copy)

            # ---- MoE stack -------------------------------------------------
            cur = xt
            for l in range(6):
                kt, mt = LAYER_KT[l], LAYER_MT[l]
                bst = sm.tile([E, 8 * P], f32r, tag="bst")
                nc.sync.dma_start(bst[:, :mt * P], b_d[l].ap())
                y = yp.tile([P, 8, B_LOC], f32, tag="y")
                for e in range(E):
                    xe = xe_pool.tile([P, 8, B_LOC], f32r, tag="xe")
                    for k in range(kt):
                        nc.vector.tensor_tensor(xe[:, k, :], cur[:, k, :],
                                                cmat[:, e, :], ALU.mult)
                    for m in range(mt):
                        wt = wt_pool.tile([P, 8, P], f32r, tag="wt")
                        nc.sync.dma_start(wt[:, :kt, :], w_d[l].ap()[e, m])
                        psum = ps.tile([P, B_LOC], f32, tag="ps")
                        if e == 0:
                            for c in range(NCH):
                                nc.tensor.matmul(psum[:, c * CH:(c + 1) * CH],
                                                 bst[:, m * P:(m + 1) * P],
                                                 coeffs[:, c * CH:(c + 1) * CH],
                                                 start=True, stop=False)
                            mm_dense(wt, xe, kt, psum, first_open=True)
                            nc.scalar.activation(y[:, m, :], psum[:], ACT.Copy)
                        else:
                            mm_dense(wt, xe, kt, psum)
                            nc.vector.tensor_tensor(y[:, m, :], psum[:], y[:, m, :],
                                                    ALU.add)
                if l < 5:
                    nxt = xp.tile([P, 8, B_LOC], f32r, tag="xt")
                    for m in range(mt):
                        elu1_evict(y[:, m, :], nxt[:, m, :])
                    cur = nxt
                else:
                    for m in range(mt):
                        nc.sync.dma_start(out_d.ap()[:, m, :], y[:, m, :])

    nc.compile()
    return nc


def _prep_w(w, pad_to=None):
    """[din, dout] -> [mt, P, kt, P] contiguous lhsT tiles (din padded)."""
    din, dout = w.shape
    if pad_to is not None and pad_to != din:
        wp = np.zeros((pad_to, dout), np.float32)
        wp[:din] = w
        w, din = wp, pad_to
    kt, mt = din // P, dout // P
    return np.ascontiguousarray(
        w.reshape(kt, P, mt, P).transpose(2, 1, 0, 3), dtype=np.float32)


def _prep_we(w, pad_to=None):
    """[E, din, dout] -> [E, mt, P, kt, P]."""
    e, din, dout = w.shape
    if pad_to is not None and pad_to != din:
        wp = np.zeros((e, pad_to, dout), np.float32)
        wp[:, :din] = w
        w, din = wp, pad_to
    kt, mt = din // P, dout // P
    return np.ascontiguousarray(
        w.reshape(e, kt, P, mt, P).transpose(0, 3, 2, 1, 4), dtype=np.float32)


def _make_in_maps(inputs):
    motion = np.asarray(inputs["motion"], np.float32)
    command = np.asarray(inputs["command"], np.float32)

    gw2 = np.asarray(inputs["g_w2"], np.float32)
    gw3 = np.asarray(inputs["g_w3"], np.float32)
    shared = {
        "gw1": _prep_w(np.asarray(inputs["g_w1"], np.float32), pad_to=IN_PAD),
        "gw2": _prep_w(gw2),
        "gw3": np.ascontiguousarray(gw3.reshape(HID // P, P, E).transpose(1, 0, 2)),
        # inter-layer activations carry elu(z)+1; fold the -1 into next biases
        "gb1": np.ascontiguousarray(np.asarray(inputs["g_b1"], np.float32).reshape(HID // P, P).T),
        "gb2": np.ascontiguousarray(
            (np.asarray(inputs["g_b2"], np.float32) - gw2.sum(0)).reshape(HID // P, P).T),
        "gb3": np.ascontiguousarray(
            (np.asarray(inputs["g_b3"], np.float32) - gw3.sum(0)).reshape(E, 1)),
    }
    for l in range(6):
        w = np.asarray(inputs[f"w{l}"], np.float32)
        bias = np.asarray(inputs[f"b{l}"], np.float32).copy()
        shared[f"w{l}"] = _prep_we(w, pad_to=IN_PAD if l == 0 else None)
        shared[f"b{l}"] = np.ascontiguousarray(bias)

    basis_np = np.zeros((E, E, P), np.float32)
    for e in range(E):
        basis_np[e, e, :] = 1.0
    shared["basis"] = basis_np
    shared["ones"] = np.ones((E, E), np.float32)

    x_cat = np.concatenate([motion, command], axis=1)
    x_pad = np.zeros((B, IN_PAD), np.float32)
    x_pad[:, :IN_DIM] = x_cat
    in_maps = []
    for c in range(N_CORES):
        xs = x_pad[c * B_LOC:(c + 1) * B_LOC]
        xt = np.ascontiguousarray(
            xs.T.reshape(IN_PAD // P, P, B_LOC).transpose(1, 0, 2))
        in_maps.append({"xt": xt, **shared})
    return in_maps


def _assemble_out(core_outs):
    outs = []
    for o in core_outs:                                    # [P, OUT/P, B_LOC]
        outs.append(o.transpose(2, 1, 0).reshape(B_LOC, OUT))
    return np.concatenate(outs, axis=0).astype(np.float32)


def kernel(**inputs):
    global _CACHED
    from concourse import bass_utils

    if _CACHED is None:
        _CACHED = _build_program()
    nc = _CACHED

    in_maps = _make_in_maps(inputs)
    res = bass_utils.run_bass_kernel_spmd(
        nc, in_maps, core_ids=list(range(N_CORES)), trace=False)
    return _assemble_out([res.results[c]["out"] for c in range(N_CORES)])
